# revision 1
# baseline (speedup 1.0000x reference)
"""Trainium2 Bass kernel for nn_CorticalGrid (predictive-coding 32x32 grid net).

Sharding: 1D over grid rows -- core k owns rows 4k..4k+3 (128 columns).
Per-step 4-neighbour halo exchange of boundary rows via AllGather through HBM,
with per-core indirect-DMA gathers of the two needed neighbour rows.

On-chip layout (per core, fp16 state, fp32 PSUM):
  partitions = feature dim x row-pair: p = o + 64*h where h = row parity in a
  row-pair block; free = 2048*t + 64*c + b  (t = row-pair block 0/1, c = grid
  col, b = batch).
  Per-column matmuls run as 2-column "pair" block-diagonal matmuls
  (K=M=128) so stationary weight loads are amortised across the pair.
"""

import sys
import numpy as np

for _p in ("/opt/trn_rl_repo", "/root/.axon_site/_ro/trn_rl_repo"):
    if _p not in sys.path:
        sys.path.append(_p)

GRID_H, GRID_W = 32, 32
OBJ_DIM, LOC_DIM = 64, 16
PATCH_H, PATCH_W = 8, 8
SENSORY_DIM = PATCH_H * PATCH_W
N_COLS = GRID_H * GRID_W
BATCH = 64
ETA = 0.05
N_CORES = 8
ROWS_PER_CORE = GRID_H // N_CORES          # 4
BLKS = ROWS_PER_CORE // 2                  # 2 row-pair blocks per core
BLK_F = GRID_W * BATCH                     # 2048 free elems per block
CORE_F = BLKS * BLK_F                      # 4096
PAIRS = BLKS * GRID_W                      # 64 pairs per core
CHUNK = 512                                # one PSUM bank (fp32)
NCHUNK = CORE_F // CHUNK                   # 8

_compiled_cache = {}


def _cnt(r, c):
    return (r > 0) + (r < GRID_H - 1) + (c > 0) + (c < GRID_W - 1)


def _build(n_steps):
    import concourse.bacc as bacc
    import concourse.mybir as mybir
    import concourse.tile as tile
    from concourse.bass import IndirectOffsetOnAxis

    F16 = mybir.dt.float16
    F32 = mybir.dt.float32
    I32 = mybir.dt.int32
    ALU = mybir.AluOpType
    ACTF = mybir.ActivationFunctionType

    nc = bacc.Bacc("TRN2", target_bir_lowering=False, debug=False,
                   num_devices=N_CORES)

    # ---- I/O ----
    d_wf_obj = nc.dram_tensor("wf_obj", [128, PAIRS * 128], F16, kind="ExternalInput")
    d_wf_loc = nc.dram_tensor("wf_loc", [32, PAIRS * 128], F16, kind="ExternalInput")
    d_wb_obj = nc.dram_tensor("wb_obj", [128, PAIRS * 128], F16, kind="ExternalInput")
    d_wb_loc = nc.dram_tensor("wb_loc", [128, PAIRS * 32], F16, kind="ExternalInput")
    d_wdiag = nc.dram_tensor("wdiag", [128, BLKS * 3 * 128], F16, kind="ExternalInput")
    d_wself = nc.dram_tensor("wself", [128, 128], F16, kind="ExternalInput")
    d_wselfloc = nc.dram_tensor("wselfloc", [32, 32], F16, kind="ExternalInput")
    d_pat = nc.dram_tensor("patches", [128, CORE_F], F16, kind="ExternalInput")
    d_offtop = nc.dram_tensor("offtop", [64, 1], I32, kind="ExternalInput")
    d_offbot = nc.dram_tensor("offbot", [64, 1], I32, kind="ExternalInput")
    d_xout = nc.dram_tensor("x_out", [128, CORE_F], F16, kind="ExternalOutput")
    d_eout = nc.dram_tensor("energy_out", [128, 2 * n_steps], F32, kind="ExternalOutput")

    with tile.TileContext(nc) as tc:
        with tc.tile_pool(name="const", bufs=1) as cp, \
             tc.tile_pool(name="state", bufs=1) as sp, \
             tc.tile_pool(name="psu", bufs=2, space="PSUM") as pu, \
             tc.tile_pool(name="psdo", bufs=2, space="PSUM") as pdo, \
             tc.tile_pool(name="psdl", bufs=2, space="PSUM") as pdl, \
             tc.tile_pool(name="dram", bufs=2, space="DRAM") as dr:

            wf_obj = cp.tile([128, PAIRS * 128], F16, tag="wf_obj")
            wf_loc = cp.tile([32, PAIRS * 128], F16, tag="wf_loc")
            wb_obj = cp.tile([128, PAIRS * 128], F16, tag="wb_obj")
            wb_loc = cp.tile([128, PAIRS * 32], F16, tag="wb_loc")
            wdiag = cp.tile([128, BLKS * 3 * 128], F16, tag="wdiag")
            wself = cp.tile([128, 128], F16, tag="wself")
            wselfloc = cp.tile([32, 32], F16, tag="wselfloc")
            pat = cp.tile([128, CORE_F], F16, tag="pat")
            offtop = cp.tile([64, 1], I32, tag="offtop")
            offbot = cp.tile([64, 1], I32, tag="offbot")
            zeros = cp.tile([64, BLK_F], F16, tag="zeros")

            for dst, src in ((wf_obj, d_wf_obj), (wf_loc, d_wf_loc),
                             (wb_obj, d_wb_obj), (wb_loc, d_wb_loc),
                             (wdiag, d_wdiag), (wself, d_wself),
                             (wselfloc, d_wselfloc), (pat, d_pat),
                             (offtop, d_offtop), (offbot, d_offbot)):
                nc.sync.dma_start(dst[:], src[:])

            x = sp.tile([128, CORE_F], F16, tag="x")
            xloc = sp.tile([32, CORE_F], F16, tag="xloc")
            xs = sp.tile([128, (BLKS + 2) * BLK_F], F16, tag="xs")
            pred = sp.tile([128, CORE_F], F16, tag="pred")
            eps = sp.tile([128, CORE_F], F16, tag="eps")
            pp = sp.tile([128, CORE_F], F16, tag="pp")
            om = sp.tile([128, CORE_F], F16, tag="om")
            g = sp.tile([128, CORE_F], F16, tag="g")
            s_ud = sp.tile([128, CORE_F], F16, tag="s_ud")
            s_lr = sp.tile([128, CORE_F], F16, tag="s_lr")
            sc = sp.tile([128, CORE_F], F16, tag="sc")
            junk = sp.tile([128, CORE_F // 2], F16, tag="junk")
            energy = sp.tile([128, 2 * n_steps], F32, tag="energy")

            nc.vector.memset(x[:], 0.0)
            nc.vector.memset(xloc[:], 0.0)
            nc.vector.memset(xs[:], 0.0)
            nc.vector.memset(zeros[:], 0.0)

            for t in range(n_steps):
                # ---- forward: u = Wf_obj@x + Wf_loc@xloc, pred = tanh(u) ----
                for c8 in range(NCHUNK):
                    u_ps = pu.tile([128, CHUNK], F32, tag="u")
                    for j in range(8):
                        p = 8 * c8 + j
                        nc.tensor.matmul(u_ps[:, 64 * j:64 * j + 64],
                                         wf_obj[:, 128 * p:128 * p + 128],
                                         x[:, 64 * p:64 * p + 64],
                                         start=(j == 0), stop=False)
                        nc.tensor.matmul(u_ps[:, 64 * j:64 * j + 64],
                                         wf_loc[:, 128 * p:128 * p + 128],
                                         xloc[:, 64 * p:64 * p + 64],
                                         start=False, stop=(j == 7))
                    nc.scalar.activation(pred[:, CHUNK * c8:CHUNK * (c8 + 1)],
                                         u_ps[:], ACTF.Tanh)

                # ---- elementwise: eps, pp, om, g, energy ----
                half = CORE_F // 2
                for h in range(2):
                    sl = slice(half * h, half * (h + 1))
                    nc.vector.tensor_tensor(eps[:, sl], pat[:, sl], pred[:, sl],
                                            op=ALU.subtract)
                    nc.gpsimd.tensor_tensor(pp[:, sl], pred[:, sl], pred[:, sl],
                                            op=ALU.mult)
                    nc.vector.tensor_scalar(om[:, sl], pp[:, sl], -1.0, 1.0,
                                            op0=ALU.mult, op1=ALU.add)
                    nc.vector.tensor_tensor(g[:, sl], eps[:, sl], om[:, sl],
                                            op=ALU.mult)
                    nc.scalar.activation(junk[:], eps[:, sl], ACTF.Square,
                                         accum_out=energy[:, 2 * t + h:2 * t + h + 1])

                # ---- neighbour sums (uses x from previous step) ----
                nc.vector.tensor_tensor(s_ud[0:64, :],
                                        xs[0:64, 0:2 * BLK_F],
                                        xs[0:64, BLK_F:3 * BLK_F], op=ALU.add)
                nc.vector.tensor_tensor(s_ud[64:128, :],
                                        xs[64:128, BLK_F:3 * BLK_F],
                                        xs[64:128, 2 * BLK_F:4 * BLK_F], op=ALU.add)
                for bb in range(BLKS):
                    base = BLK_F * bb
                    nc.vector.tensor_tensor(s_lr[:, base + 64:base + BLK_F - 64],
                                            x[:, base:base + BLK_F - 128],
                                            x[:, base + 128:base + BLK_F],
                                            op=ALU.add)
                    nc.vector.tensor_copy(s_lr[:, base:base + 64],
                                          x[:, base + 64:base + 128])
                    nc.vector.tensor_copy(s_lr[:, base + BLK_F - 64:base + BLK_F],
                                          x[:, base + BLK_F - 128:base + BLK_F - 64])
                nc.gpsimd.tensor_tensor(sc[:], s_ud[:], s_lr[:], op=ALU.add)

                # ---- backward + state update ----
                for c8 in range(NCHUNK):
                    dxo_ps = pdo.tile([128, CHUNK], F32, tag="dxo")
                    dxl_ps = pdl.tile([32, CHUNK], F32, tag="dxl")
                    for j in range(8):
                        p = 8 * c8 + j
                        nc.tensor.matmul(dxo_ps[:, 64 * j:64 * j + 64],
                                         wb_obj[:, 128 * p:128 * p + 128],
                                         g[:, 64 * p:64 * p + 64],
                                         start=(j == 0), stop=False)
                    bb, q = c8 // 4, c8 % 4
                    base = BLK_F * bb + CHUNK * q
                    if q == 0:
                        groups = [(0, 64, 0), (64, 448, 1)]
                    elif q == 3:
                        groups = [(0, 448, 1), (448, 64, 2)]
                    else:
                        groups = [(0, 512, 1)]
                    for off, n, gr in groups:
                        nc.tensor.matmul(dxo_ps[:, off:off + n],
                                         wdiag[:, 384 * bb + 128 * gr:384 * bb + 128 * gr + 128],
                                         sc[:, base + off:base + off + n],
                                         start=False, stop=False)
                    nc.tensor.matmul(dxo_ps[:, :],
                                     wself[:, :],
                                     x[:, CHUNK * c8:CHUNK * (c8 + 1)],
                                     start=False, stop=True)
                    for j in range(8):
                        p = 8 * c8 + j
                        nc.tensor.matmul(dxl_ps[:, 64 * j:64 * j + 64],
                                         wb_loc[:, 32 * p:32 * p + 32],
                                         g[:, 64 * p:64 * p + 64],
                                         start=(j == 0), stop=False)
                    nc.tensor.matmul(dxl_ps[:, :],
                                     wselfloc[:, :],
                                     xloc[:, CHUNK * c8:CHUNK * (c8 + 1)],
                                     start=False, stop=True)
                    nc.scalar.activation(x[:, CHUNK * c8:CHUNK * (c8 + 1)],
                                         dxo_ps[:], ACTF.Copy)
                    nc.vector.tensor_copy(xloc[:, CHUNK * c8:CHUNK * (c8 + 1)],
                                          dxl_ps[:])

                # ---- halo exchange + shifted-copy maintenance for next step ----
                if t < n_steps - 1:
                    ag_in = dr.tile([128, BLK_F], F16, tag="ag_in")
                    ag_out = dr.tile([(2 * N_CORES + 1) * 64, BLK_F], F16, tag="ag_out")
                    nc.sync.dma_start(ag_in[0:64, :], x[0:64, 0:BLK_F])
                    nc.sync.dma_start(ag_in[64:128, :], x[64:128, BLK_F:2 * BLK_F])
                    nc.sync.dma_start(ag_out[2 * N_CORES * 64:, :], zeros[:])
                    nc.gpsimd.collective_compute(
                        "AllGather", ALU.bypass,
                        replica_groups=[list(range(N_CORES))],
                        ins=[ag_in[:]],
                        outs=[ag_out[0:2 * N_CORES * 64, :]],
                    )
                    nc.sync.dma_start(xs[0:64, BLK_F:3 * BLK_F], x[64:128, :])
                    nc.sync.dma_start(xs[64:128, BLK_F:3 * BLK_F], x[0:64, :])
                    nc.gpsimd.indirect_dma_start(
                        out=xs[0:64, 0:BLK_F], out_offset=None,
                        in_=ag_out[:],
                        in_offset=IndirectOffsetOnAxis(ap=offtop[:, :1], axis=0))
                    nc.gpsimd.indirect_dma_start(
                        out=xs[64:128, 3 * BLK_F:4 * BLK_F], out_offset=None,
                        in_=ag_out[:],
                        in_offset=IndirectOffsetOnAxis(ap=offbot[:, :1], axis=0))

            nc.sync.dma_start(d_xout[:], x[:])
            nc.sync.dma_start(d_eout[:], energy[:])

    nc.compile()
    return nc


def _get_compiled(n_steps):
    if n_steps not in _compiled_cache:
        _compiled_cache[n_steps] = _build(n_steps)
    return _compiled_cache[n_steps]


def _prep_core(k, patches, W_obj, W_loc):
    """Build per-core constant tensors. patches: (N_COLS, BATCH, SENSORY)."""
    f16 = np.float16
    eta = np.float32(ETA)
    pat = np.zeros((128, CORE_F), f16)
    wf_obj = np.zeros((128, PAIRS * 128), f16)
    wf_loc = np.zeros((32, PAIRS * 128), f16)
    wb_obj = np.zeros((128, PAIRS * 128), f16)
    wb_loc = np.zeros((128, PAIRS * 32), f16)
    for t in range(BLKS):
        for c in range(GRID_W):
            p = t * GRID_W + c
            for h in range(2):
                row = 4 * k + 2 * t + h
                n = row * GRID_W + c
                ps = slice(64 * h, 64 * h + 64)
                pat[ps, BLK_F * t + 64 * c:BLK_F * t + 64 * c + 64] = \
                    patches[n].T.astype(f16)
                wf_obj[ps, 128 * p + 64 * h:128 * p + 64 * h + 64] = \
                    W_obj[n].astype(f16)
                wf_loc[16 * h:16 * h + 16, 128 * p + 64 * h:128 * p + 64 * h + 64] = \
                    W_loc[n].astype(f16)
                wb_obj[ps, 128 * p + 64 * h:128 * p + 64 * h + 64] = \
                    (eta * W_obj[n]).T.astype(f16)
                wb_loc[ps, 32 * p + 16 * h:32 * p + 16 * h + 16] = \
                    (eta * W_loc[n]).T.astype(f16)

    wdiag = np.zeros((128, BLKS * 3 * 128), f16)
    for t in range(BLKS):
        for gi, cc in enumerate((0, GRID_W // 2, GRID_W - 1)):
            for p in range(128):
                row = 4 * k + 2 * t + (p // 64)
                wdiag[p, 384 * t + 128 * gi + p] = np.float16(eta / _cnt(row, cc))
    wself = (np.float16(1.0 - ETA) * np.eye(128, dtype=f16)).astype(f16)
    wselfloc = np.eye(32, dtype=f16)

    offtop = np.zeros((64, 1), np.int32)
    offbot = np.zeros((64, 1), np.int32)
    prng = np.arange(64, dtype=np.int32)
    zbase = 2 * N_CORES * 64
    offtop[:, 0] = (128 * (k - 1) + 64 + prng) if k > 0 else (zbase + prng)
    offbot[:, 0] = (128 * (k + 1) + prng) if k < N_CORES - 1 else (zbase + prng)

    return {"wf_obj": wf_obj, "wf_loc": wf_loc, "wb_obj": wb_obj,
            "wb_loc": wb_loc, "wdiag": wdiag, "wself": wself,
            "wselfloc": wselfloc, "patches": pat,
            "offtop": offtop, "offbot": offbot}


def _slice_patches(global_input):
    B = global_input.shape[0]
    img = global_input.reshape(B, GRID_H, PATCH_H, GRID_W, PATCH_W)
    return img.transpose(1, 3, 0, 2, 4).reshape(N_COLS, B, SENSORY_DIM)


def kernel(global_input, W_obj, W_loc, steps, _want_results=False, _trace=False):
    from concourse import bass_utils

    n_steps = int(np.asarray(steps))
    gi = np.asarray(global_input, dtype=np.float32)
    W_obj = np.asarray(W_obj, dtype=np.float32)
    W_loc = np.asarray(W_loc, dtype=np.float32)

    nc = _get_compiled(n_steps)
    patches = _slice_patches(gi)
    in_maps = [_prep_core(k, patches, W_obj, W_loc) for k in range(N_CORES)]
    res = bass_utils.run_bass_kernel_spmd(
        nc, in_maps, core_ids=list(range(N_CORES)), trace=_trace)

    x_obj = np.zeros((N_COLS, BATCH, OBJ_DIM), np.float32)
    energy = np.zeros((n_steps,), np.float32)
    for k in range(N_CORES):
        r = res.results[k]
        a = r["x_out"].astype(np.float32).reshape(2, 64, BLKS, GRID_W, BATCH)
        for t in range(BLKS):
            for h in range(2):
                row = 4 * k + 2 * t + h
                # a[h, o, t, c, b] -> (c, b, o)
                x_obj[row * GRID_W:(row + 1) * GRID_W] = \
                    a[h, :, t, :, :].transpose(1, 2, 0)
        energy += 0.5 * r["energy_out"].reshape(128, n_steps, 2).sum(axis=(0, 2))

    if _want_results:
        return (x_obj, energy), res
    return x_obj, energy


# revision 3
# speedup vs baseline: 1.2732x; 1.2732x over previous
"""Trainium2 Bass kernel for nn_CorticalGrid (predictive-coding 32x32 grid net).

Sharding: 1D over grid rows -- core k owns rows 4k..4k+3 (128 columns).
Per-step 4-neighbour halo exchange of boundary rows via AllGather through HBM,
with per-core indirect-DMA gathers of the two needed neighbour rows.

On-chip layout (per core, fp16 state, fp32 PSUM):
  partitions = feature dim x row-pair: p = o + 64*h where h = row parity in a
  row-pair block; free = 2048*t + 64*c + b  (t = row-pair block 0/1, c = grid
  col, b = batch).
  Per-column matmuls run as 2-column "pair" block-diagonal matmuls
  (K=M=128) so stationary weight loads are amortised across the pair.
"""

import sys
import numpy as np

for _p in ("/opt/trn_rl_repo", "/root/.axon_site/_ro/trn_rl_repo"):
    if _p not in sys.path:
        sys.path.append(_p)

GRID_H, GRID_W = 32, 32
OBJ_DIM, LOC_DIM = 64, 16
PATCH_H, PATCH_W = 8, 8
SENSORY_DIM = PATCH_H * PATCH_W
N_COLS = GRID_H * GRID_W
BATCH = 64
ETA = 0.05
N_CORES = 8
ROWS_PER_CORE = GRID_H // N_CORES          # 4
BLKS = ROWS_PER_CORE // 2                  # 2 row-pair blocks per core
BLK_F = GRID_W * BATCH                     # 2048 free elems per block
CORE_F = BLKS * BLK_F                      # 4096
PAIRS = BLKS * GRID_W                      # 64 pairs per core
CHUNK = 512                                # one PSUM bank (fp32)
NCHUNK = CORE_F // CHUNK                   # 8

_compiled_cache = {}


def _cnt(r, c):
    return (r > 0) + (r < GRID_H - 1) + (c > 0) + (c < GRID_W - 1)


def _build(n_steps):
    import concourse.bacc as bacc
    import concourse.mybir as mybir
    import concourse.tile as tile
    from concourse.bass import IndirectOffsetOnAxis

    F16 = mybir.dt.float16
    F32 = mybir.dt.float32
    I32 = mybir.dt.int32
    ALU = mybir.AluOpType
    ACTF = mybir.ActivationFunctionType

    nc = bacc.Bacc("TRN2", target_bir_lowering=False, debug=False,
                   num_devices=N_CORES)

    # ---- I/O ----
    d_wf_obj = nc.dram_tensor("wf_obj", [128, PAIRS * 128], F16, kind="ExternalInput")
    d_wf_loc = nc.dram_tensor("wf_loc", [32, PAIRS * 128], F16, kind="ExternalInput")
    d_wb_obj = nc.dram_tensor("wb_obj", [128, PAIRS * 128], F16, kind="ExternalInput")
    d_wb_loc = nc.dram_tensor("wb_loc", [128, PAIRS * 32], F16, kind="ExternalInput")
    d_wdiag = nc.dram_tensor("wdiag", [128, BLKS * 3 * 128], F16, kind="ExternalInput")
    d_wself = nc.dram_tensor("wself", [128, 128], F16, kind="ExternalInput")
    d_wselfloc = nc.dram_tensor("wselfloc", [32, 32], F16, kind="ExternalInput")
    d_pat = nc.dram_tensor("patches", [128, CORE_F], F16, kind="ExternalInput")
    d_mask = nc.dram_tensor("mask", [128, 1], F16, kind="ExternalInput")
    d_offtop = nc.dram_tensor("offtop", [64, 1], I32, kind="ExternalInput")
    d_offbot = nc.dram_tensor("offbot", [64, 1], I32, kind="ExternalInput")
    d_xout = nc.dram_tensor("x_out", [128, CORE_F], F16, kind="ExternalOutput")
    d_eout = nc.dram_tensor("energy_out", [128, 2 * n_steps], F32, kind="ExternalOutput")

    with tile.TileContext(nc) as tc:
        with tc.tile_pool(name="const", bufs=1) as cp, \
             tc.tile_pool(name="state", bufs=1) as sp, \
             tc.tile_pool(name="psu", bufs=3, space="PSUM") as pu, \
             tc.tile_pool(name="psdo", bufs=2, space="PSUM") as pdo, \
             tc.tile_pool(name="psdl", bufs=2, space="PSUM") as pdl, \
             tc.tile_pool(name="dram", bufs=2, space="DRAM") as dr:

            wf_obj = cp.tile([128, PAIRS * 128], F16, tag="wf_obj")
            wf_loc = cp.tile([32, PAIRS * 128], F16, tag="wf_loc")
            wb_obj = cp.tile([128, PAIRS * 128], F16, tag="wb_obj")
            wb_loc = cp.tile([128, PAIRS * 32], F16, tag="wb_loc")
            wdiag = cp.tile([128, BLKS * 3 * 128], F16, tag="wdiag")
            wself = cp.tile([128, 128], F16, tag="wself")
            wselfloc = cp.tile([32, 32], F16, tag="wselfloc")
            pat = cp.tile([128, CORE_F], F16, tag="pat")
            mask = cp.tile([128, 1], F16, tag="mask")
            offtop = cp.tile([64, 1], I32, tag="offtop")
            offbot = cp.tile([64, 1], I32, tag="offbot")

            for dst, src in ((wf_obj, d_wf_obj), (wf_loc, d_wf_loc),
                             (wb_obj, d_wb_obj), (wb_loc, d_wb_loc),
                             (wdiag, d_wdiag), (wself, d_wself),
                             (wselfloc, d_wselfloc), (pat, d_pat),
                             (mask, d_mask),
                             (offtop, d_offtop), (offbot, d_offbot)):
                nc.sync.dma_start(dst[:], src[:])

            x = sp.tile([128, CORE_F], F16, tag="x")
            xloc = sp.tile([32, CORE_F], F16, tag="xloc")
            xs = sp.tile([128, (BLKS + 2) * BLK_F], F16, tag="xs")
            pred = sp.tile([128, CORE_F], F16, tag="pred")
            eps = sp.tile([128, CORE_F], F16, tag="eps")
            pp = sp.tile([128, CORE_F], F16, tag="pp")
            om = sp.tile([128, CORE_F], F16, tag="om")
            g = sp.tile([128, CORE_F], F16, tag="g")
            s_ud = sp.tile([128, CORE_F], F16, tag="s_ud")
            s_lr = sp.tile([128, CORE_F], F16, tag="s_lr")
            sc = sp.tile([128, CORE_F], F16, tag="sc")
            junk = sp.tile([128, CORE_F // 2], F16, tag="junk")
            energy = sp.tile([128, 2 * n_steps], F32, tag="energy")

            nc.vector.memset(x[:], 0.0)
            nc.vector.memset(xloc[:], 0.0)
            nc.vector.memset(xs[:], 0.0)

            for t in range(n_steps):
                # ---- forward: u = Wf_obj@x + Wf_loc@xloc, pred = tanh(u) ----
                for c8 in range(NCHUNK):
                    u_ps = pu.tile([128, CHUNK], F32, tag="u")
                    for j in range(8):
                        p = 8 * c8 + j
                        nc.tensor.matmul(u_ps[:, 64 * j:64 * j + 64],
                                         wf_obj[:, 128 * p:128 * p + 128],
                                         x[:, 64 * p:64 * p + 64],
                                         start=(j == 0), stop=False)
                        nc.tensor.matmul(u_ps[:, 64 * j:64 * j + 64],
                                         wf_loc[:, 128 * p:128 * p + 128],
                                         xloc[:, 64 * p:64 * p + 64],
                                         start=False, stop=(j == 7))
                    nc.scalar.activation(pred[:, CHUNK * c8:CHUNK * (c8 + 1)],
                                         u_ps[:], ACTF.Tanh)

                # ---- elementwise: eps, pp, om, g, energy ----
                half = CORE_F // 2
                for h in range(2):
                    sl = slice(half * h, half * (h + 1))
                    nc.vector.tensor_tensor(eps[:, sl], pat[:, sl], pred[:, sl],
                                            op=ALU.subtract)
                    nc.vector.tensor_tensor(pp[:, sl], pred[:, sl], pred[:, sl],
                                            op=ALU.mult)
                    nc.vector.tensor_scalar(om[:, sl], pp[:, sl], -1.0, 1.0,
                                            op0=ALU.mult, op1=ALU.add)
                    nc.vector.tensor_tensor(g[:, sl], eps[:, sl], om[:, sl],
                                            op=ALU.mult)
                    nc.scalar.activation(junk[:], eps[:, sl], ACTF.Square,
                                         accum_out=energy[:, 2 * t + h:2 * t + h + 1])

                # ---- neighbour sums (uses x from previous step) ----
                nc.vector.scalar_tensor_tensor(
                    s_ud[0:64, 0:BLK_F], xs[0:64, 0:BLK_F], mask[0:64, :],
                    xs[0:64, BLK_F:2 * BLK_F], op0=ALU.mult, op1=ALU.add)
                nc.vector.tensor_tensor(s_ud[0:64, BLK_F:2 * BLK_F],
                                        xs[0:64, BLK_F:2 * BLK_F],
                                        xs[0:64, 2 * BLK_F:3 * BLK_F], op=ALU.add)
                nc.vector.tensor_tensor(s_ud[64:128, 0:BLK_F],
                                        xs[64:128, BLK_F:2 * BLK_F],
                                        xs[64:128, 2 * BLK_F:3 * BLK_F], op=ALU.add)
                nc.vector.scalar_tensor_tensor(
                    s_ud[64:128, BLK_F:2 * BLK_F], xs[64:128, 3 * BLK_F:4 * BLK_F],
                    mask[64:128, :], xs[64:128, 2 * BLK_F:3 * BLK_F],
                    op0=ALU.mult, op1=ALU.add)
                for bb in range(BLKS):
                    base = BLK_F * bb
                    nc.vector.tensor_tensor(s_lr[:, base + 64:base + BLK_F - 64],
                                            x[:, base:base + BLK_F - 128],
                                            x[:, base + 128:base + BLK_F],
                                            op=ALU.add)
                    nc.vector.tensor_copy(s_lr[:, base:base + 64],
                                          x[:, base + 64:base + 128])
                    nc.vector.tensor_copy(s_lr[:, base + BLK_F - 64:base + BLK_F],
                                          x[:, base + BLK_F - 128:base + BLK_F - 64])
                nc.vector.tensor_tensor(sc[:], s_ud[:], s_lr[:], op=ALU.add)

                # ---- backward + state update ----
                for c8 in range(NCHUNK):
                    dxo_ps = pdo.tile([128, CHUNK], F32, tag="dxo")
                    dxl_ps = pdl.tile([32, CHUNK], F32, tag="dxl")
                    for j in range(8):
                        p = 8 * c8 + j
                        nc.tensor.matmul(dxo_ps[:, 64 * j:64 * j + 64],
                                         wb_obj[:, 128 * p:128 * p + 128],
                                         g[:, 64 * p:64 * p + 64],
                                         start=(j == 0), stop=False)
                    bb, q = c8 // 4, c8 % 4
                    base = BLK_F * bb + CHUNK * q
                    if q == 0:
                        groups = [(0, 64, 0), (64, 448, 1)]
                    elif q == 3:
                        groups = [(0, 448, 1), (448, 64, 2)]
                    else:
                        groups = [(0, 512, 1)]
                    for off, n, gr in groups:
                        nc.tensor.matmul(dxo_ps[:, off:off + n],
                                         wdiag[:, 384 * bb + 128 * gr:384 * bb + 128 * gr + 128],
                                         sc[:, base + off:base + off + n],
                                         start=False, stop=False)
                    nc.tensor.matmul(dxo_ps[:, :],
                                     wself[:, :],
                                     x[:, CHUNK * c8:CHUNK * (c8 + 1)],
                                     start=False, stop=True)
                    for j in range(8):
                        p = 8 * c8 + j
                        nc.tensor.matmul(dxl_ps[:, 64 * j:64 * j + 64],
                                         wb_loc[:, 32 * p:32 * p + 32],
                                         g[:, 64 * p:64 * p + 64],
                                         start=(j == 0), stop=False)
                    nc.tensor.matmul(dxl_ps[:, :],
                                     wselfloc[:, :],
                                     xloc[:, CHUNK * c8:CHUNK * (c8 + 1)],
                                     start=False, stop=True)
                    nc.scalar.activation(x[:, CHUNK * c8:CHUNK * (c8 + 1)],
                                         dxo_ps[:], ACTF.Copy)
                    nc.vector.tensor_copy(xloc[:, CHUNK * c8:CHUNK * (c8 + 1)],
                                          dxl_ps[:])

                # ---- halo exchange + shifted-copy maintenance for next step ----
                if t < n_steps - 1:
                    ag_in = dr.tile([128, BLK_F], F16, tag="ag_in")
                    ag_out = dr.tile([2 * N_CORES * 64, BLK_F], F16, tag="ag_out", addr_space="Shared")
                    nc.sync.dma_start(ag_in[0:64, :], x[0:64, 0:BLK_F])
                    nc.sync.dma_start(ag_in[64:128, :], x[64:128, BLK_F:2 * BLK_F])
                    nc.gpsimd.collective_compute(
                        "AllGather", ALU.bypass,
                        replica_groups=[list(range(N_CORES))],
                        ins=[ag_in[:]],
                        outs=[ag_out[0:2 * N_CORES * 64, :]],
                    )
                    nc.sync.dma_start(xs[0:64, BLK_F:3 * BLK_F], x[64:128, :])
                    nc.sync.dma_start(xs[64:128, BLK_F:3 * BLK_F], x[0:64, :])
                    nc.gpsimd.indirect_dma_start(
                        out=xs[0:64, 0:BLK_F], out_offset=None,
                        in_=ag_out[:],
                        in_offset=IndirectOffsetOnAxis(ap=offtop[:, :1], axis=0))
                    nc.gpsimd.indirect_dma_start(
                        out=xs[64:128, 3 * BLK_F:4 * BLK_F], out_offset=None,
                        in_=ag_out[:],
                        in_offset=IndirectOffsetOnAxis(ap=offbot[:, :1], axis=0))

            nc.sync.dma_start(d_xout[:], x[:])
            nc.sync.dma_start(d_eout[:], energy[:])

    nc.compile()
    return nc


def _get_compiled(n_steps):
    if n_steps not in _compiled_cache:
        _compiled_cache[n_steps] = _build(n_steps)
    return _compiled_cache[n_steps]


def _prep_core(k, patches, W_obj, W_loc):
    """Build per-core constant tensors. patches: (N_COLS, BATCH, SENSORY)."""
    f16 = np.float16
    eta = np.float32(ETA)
    pat = np.zeros((128, CORE_F), f16)
    wf_obj = np.zeros((128, PAIRS * 128), f16)
    wf_loc = np.zeros((32, PAIRS * 128), f16)
    wb_obj = np.zeros((128, PAIRS * 128), f16)
    wb_loc = np.zeros((128, PAIRS * 32), f16)
    for t in range(BLKS):
        for c in range(GRID_W):
            p = t * GRID_W + c
            for h in range(2):
                row = 4 * k + 2 * t + h
                n = row * GRID_W + c
                ps = slice(64 * h, 64 * h + 64)
                pat[ps, BLK_F * t + 64 * c:BLK_F * t + 64 * c + 64] = \
                    patches[n].T.astype(f16)
                wf_obj[ps, 128 * p + 64 * h:128 * p + 64 * h + 64] = \
                    W_obj[n].astype(f16)
                wf_loc[16 * h:16 * h + 16, 128 * p + 64 * h:128 * p + 64 * h + 64] = \
                    W_loc[n].astype(f16)
                wb_obj[ps, 128 * p + 64 * h:128 * p + 64 * h + 64] = \
                    (eta * W_obj[n]).T.astype(f16)
                wb_loc[ps, 32 * p + 16 * h:32 * p + 16 * h + 16] = \
                    (eta * W_loc[n]).T.astype(f16)

    wdiag = np.zeros((128, BLKS * 3 * 128), f16)
    for t in range(BLKS):
        for gi, cc in enumerate((0, GRID_W // 2, GRID_W - 1)):
            for p in range(128):
                row = 4 * k + 2 * t + (p // 64)
                wdiag[p, 384 * t + 128 * gi + p] = np.float16(eta / _cnt(row, cc))
    wself = (np.float16(1.0 - ETA) * np.eye(128, dtype=f16)).astype(f16)
    wselfloc = np.eye(32, dtype=f16)

    offtop = np.zeros((64, 1), np.int32)
    offbot = np.zeros((64, 1), np.int32)
    prng = np.arange(64, dtype=np.int32)
    offtop[:, 0] = (128 * (k - 1) + 64 + prng) if k > 0 else (128 * k + prng)
    offbot[:, 0] = (128 * (k + 1) + prng) if k < N_CORES - 1 else (128 * k + prng)
    mask = np.zeros((128, 1), np.float16)
    mask[0:64] = 1.0 if k > 0 else 0.0
    mask[64:128] = 1.0 if k < N_CORES - 1 else 0.0

    return {"wf_obj": wf_obj, "wf_loc": wf_loc, "wb_obj": wb_obj,
            "wb_loc": wb_loc, "wdiag": wdiag, "wself": wself,
            "wselfloc": wselfloc, "patches": pat, "mask": mask,
            "offtop": offtop, "offbot": offbot}


def _slice_patches(global_input):
    B = global_input.shape[0]
    img = global_input.reshape(B, GRID_H, PATCH_H, GRID_W, PATCH_W)
    return img.transpose(1, 3, 0, 2, 4).reshape(N_COLS, B, SENSORY_DIM)


def kernel(global_input, W_obj, W_loc, steps, _want_results=False, _trace=False):
    from concourse import bass_utils

    n_steps = int(np.asarray(steps))
    gi = np.asarray(global_input, dtype=np.float32)
    W_obj = np.asarray(W_obj, dtype=np.float32)
    W_loc = np.asarray(W_loc, dtype=np.float32)

    nc = _get_compiled(n_steps)
    patches = _slice_patches(gi)
    in_maps = [_prep_core(k, patches, W_obj, W_loc) for k in range(N_CORES)]
    res = bass_utils.run_bass_kernel_spmd(
        nc, in_maps, core_ids=list(range(N_CORES)), trace=_trace)

    x_obj = np.zeros((N_COLS, BATCH, OBJ_DIM), np.float32)
    energy = np.zeros((n_steps,), np.float32)
    for k in range(N_CORES):
        r = res.results[k]
        a = r["x_out"].astype(np.float32).reshape(2, 64, BLKS, GRID_W, BATCH)
        for t in range(BLKS):
            for h in range(2):
                row = 4 * k + 2 * t + h
                # a[h, o, t, c, b] -> (c, b, o)
                x_obj[row * GRID_W:(row + 1) * GRID_W] = \
                    a[h, :, t, :, :].transpose(1, 2, 0)
        energy += 0.5 * r["energy_out"].reshape(128, n_steps, 2).sum(axis=(0, 2))

    if _want_results:
        return (x_obj, energy), res
    return x_obj, energy
